# revision 44
# baseline (speedup 1.0000x reference)
"""Single-head attention (B=8, S=2048, E=1024, H=128) with softmax + deterministic
dropout, data-parallel over batch across 8 NeuronCores (one batch element per core).

Layout ("transposed attention"), engineered so the ACT engine (which does the
4M-element exp() per core, the hard throughput floor at ~27us) never bubbles:

  - host ships x with the projection weights CONCATENATED per e-chunk row:
    xw8 fp8e4m3 [128, NE, S+2H] = x8 rows ++ wq8 ++ wk8 (weights pre-scaled by
    32 for fp8 subnormal avoidance, folded into the exp scale), and
    xwv fp16 [128, NE, S+H] = xT rows ++ wv.  One contiguous stream per dtype:
    no separate (slow) small weight DMAs, and the first q,k matmul can start
    as soon as the first e-pair lands.
  - q,k projections are fp8 DoubleRow matmuls (K=256/instruction, 2x PE rate);
    v projection is fp16.
  - dropout mask shipped {0,1} fp16 (keeps the wide DVE multiply in 2x 16-bit
    mode; the 1/(1-p) factor is folded into the denominator select matmul).
    The mask/xwv/output queue (Sync) is GATED behind the first q,k eviction
    via a dummy copy into the first mask tile, so it cannot steal HBM
    bandwidth from the projection-critical fp8 stream at startup.
  - attention is one flat stream of t-pairs: per pair one 2-bank PSUM tile
    takes two QK matmuls, one wide ACT exp -> expT fp16, one wide DVE mask
    multiply.  Each s-group's post-work (denominator waves, AV matmuls,
    normalize) is INTERLEAVED into the NEXT s-group's pair emission, so the
    PE always has exp-feeding pair matmuls in flight and ACT stays saturated
    across s-group boundaries.  The v projection is interleaved into
    s-group 0's pairs the same way, using the out/den PSUM banks (idle until
    the first AV matmul) for its accumulators and transpose scratch.
  - denominator: M=1 ones-matmuls per expT chunk in waves of 4 packed into
    distinct PE column groups (tile_position), emitted when operands are
    long-ready so they issue back-to-back and overlap in the PE array.
  - normalize: den partials in PSUM rows {0,32,64,96}; one select matmul sums
    and broadcasts 0.9*den[s]; reciprocal_approx_fast + one DVE multiply
    rescale out^T during eviction.  Output stays [H, S] fp16; the host
    transposes back to [S, H] fp32.

PSUM (8 banks): 3 x [128,2,512] QK tiles (6) + out (1) + den (1), with the v
projection and its transpose scratch time-sharing out/den before the AVs.
"""

import sys

for _p in ("/opt/trn_rl_repo",):
    if _p not in sys.path:
        sys.path.append(_p)

import numpy as np
import ml_dtypes

B, S, E, H = 8, 2048, 1024, 128
DROP_P = 0.1
P = 128
W_SCALE = 32.0  # host premultiplies wq/wk by this before fp8 cast

_program_cache = {}


def _build_program(S=S, E=E):
    key = (S, E)
    if key in _program_cache:
        return _program_cache[key]
    NT = S // P       # t-chunks (16)
    NE = E // P       # e-chunks (8)
    NEP = NE // 2     # e-pairs for DoubleRow (4)
    SG = 512          # s-group width (one fp32 PSUM bank)
    NSG = S // SG     # 4
    NPAIR = NT // 2   # t-pairs per s-group (8)
    W8 = S + 2 * H    # xw8 row length
    WV = S + H        # xwv row length

    import concourse.bass as bass  # noqa: F401
    import concourse.mybir as mybir
    import concourse.tile as tile
    from concourse import bacc
    from concourse.masks import make_identity

    f32 = mybir.dt.float32
    f16 = mybir.dt.float16
    f8 = mybir.dt.float8e4
    Exp = mybir.ActivationFunctionType.Exp
    DR = mybir.MatmulPerfMode.DoubleRow
    SCALE = float(E) ** -0.5 / (W_SCALE * W_SCALE)

    nc = bacc.Bacc("TRN2", target_bir_lowering=False, debug=False)
    xw8_d = nc.dram_tensor("xw8", [P, NE, W8], f8, kind="ExternalInput").ap()
    xwv_d = nc.dram_tensor("xwv", [P, NE, WV], f16, kind="ExternalInput").ap()
    keep_d = nc.dram_tensor("keep", [P, NT, S], f16, kind="ExternalInput").ap()
    outT_d = nc.dram_tensor("outT", [H, S], f16, kind="ExternalOutput").ap()

    with tile.TileContext(nc) as tc:
        with (
            tc.tile_pool(name="consts", bufs=1) as consts,
            tc.tile_pool(name="xw", bufs=1) as xw_pool,
            tc.tile_pool(name="keep_pool", bufs=2) as keep_pool,
        ):
            identity16 = consts.tile([P, P], f16)
            make_identity(nc, identity16)
            ones_t = consts.tile([P, 1], f16)
            nc.vector.memset(ones_t, 1.0)
            sel128 = consts.tile([P, P], f16)
            nc.vector.memset(sel128, 0.0)
            for j in range(4):
                nc.vector.memset(sel128[32 * j:32 * j + 1, :], 1.0 - DROP_P)

            xw8_sb = xw_pool.tile([P, NE, W8], f8, tag="xw8", name="xw8")
            xwv_sb = xw_pool.tile([P, NE, WV], f16, tag="xwv", name="xwv")
            qkT_sb = xw_pool.tile([P, 2, S], f16, tag="qkT", name="qkT")
            vT_sb = xw_pool.tile([P, S], f16, tag="vT", name="vT")
            v_sb = xw_pool.tile([P, NT, H], f16, tag="v", name="v")

            # qAct (scalar) queue: the projection-critical fp8 stream, then
            # the tail of xwv.  qSP starts with the head of xwv (small, needed
            # early by the v projection) — the mask fetches behind it are
            # gated below.
            for ep in range(NEP):
                nc.scalar.dma_start(
                    xw8_sb[:, 2 * ep:2 * ep + 2, :], xw8_d[:, 2 * ep:2 * ep + 2, :]
                )
            for ep in (2, 3):
                nc.scalar.dma_start(
                    xwv_sb[:, 2 * ep:2 * ep + 2, :], xwv_d[:, 2 * ep:2 * ep + 2, :]
                )

            keeps = {}

            def fetch_keep(sg, engine, half=None):
                if sg not in keeps:
                    keeps[sg] = keep_pool.tile([P, NT, SG], f16, tag="keep",
                                               name=f"keep{sg}")
                t_ = keeps[sg]
                sl = slice(sg * SG, (sg + 1) * SG)
                if half is None:
                    engine.dma_start(t_, keep_d[:, :, sl])
                else:
                    rows = slice(0, NT // 2) if half == 0 else slice(NT // 2, NT)
                    engine.dma_start(t_[:, rows, :], keep_d[:, rows, sl])

            # -------- q,k projections: fp8 DoubleRow, e-pair-major --------
            with tc.tile_pool(name="proj_ps", bufs=8, space="PSUM") as proj_ps:
                ps_qk = [
                    proj_ps.tile([P, SG], f32, tag=f"pqk{j}{c}",
                                 name=f"pqk{j}{c}", bufs=1)
                    for j in range(2) for c in range(NSG)
                ]
                for ep in range(NEP):
                    for j in range(2):
                        for c in range(NSG):
                            nc.tensor.matmul(
                                ps_qk[j * NSG + c],
                                xw8_sb[:, 2 * ep:2 * ep + 2,
                                       S + j * H:S + (j + 1) * H],
                                xw8_sb[:, 2 * ep:2 * ep + 2, c * SG:(c + 1) * SG],
                                start=(ep == 0),
                                stop=(ep == NEP - 1),
                                perf_mode=DR,
                            )
                for j in range(2):
                    for c in range(NSG):
                        nc.any.tensor_copy(
                            qkT_sb[:, j, c * SG:(c + 1) * SG],
                            ps_qk[j * NSG + c],
                        )

            # Gate the Sync queue behind the first q,k eviction: emitted
            # AFTER the eviction, this copy reads qkT (RAW dep) and writes a
            # corner of the first mask tile, so the mask DMA — and everything
            # queued after it on the Sync queue — cannot steal HBM bandwidth
            # from the projection-critical fp8 stream.
            # qSP queue: first mask half, xwv head (the v projection wants
            # it early), then the remaining mask fetches.  Empirically this
            # ordering beats gating the mask stream behind the projection:
            # the HBM arbiter shares bandwidth well enough, and a late mask
            # stalls the DVE multiply FIFO, which cascades.
            fetch_keep(0, nc.sync, half=0)
            for ep in (0, 1):
                nc.sync.dma_start(
                    xwv_sb[:, 2 * ep:2 * ep + 2, :], xwv_d[:, 2 * ep:2 * ep + 2, :]
                )
            fetch_keep(0, nc.sync, half=1)
            fetch_keep(1, nc.sync)
            fetch_keep(2, nc.sync)
            fetch_keep(3, nc.sync)

            # -------- flat attention pipeline --------
            with (
                tc.tile_pool(name="att_ps", bufs=3, space="PSUM") as att_ps,
                tc.tile_pool(name="acc_ps", bufs=1, space="PSUM") as acc_ps,
                tc.tile_pool(name="work", bufs=2) as work_pool,
            ):
                state = {}

                def sg_state(sg):
                    if sg not in state:
                        state[sg] = dict(
                            out=acc_ps.tile([P, SG], f32, tag="out",
                                            name=f"out{sg}"),
                            den=acc_ps.tile([P, SG], f32, tag="den",
                                            name=f"den{sg}"),
                            expTs={}, attds={},
                        )
                    return state[sg]

                def emit_pair(sg, i):
                    st = sg_state(sg)
                    s_sl = slice(sg * SG, (sg + 1) * SG)
                    ps = att_ps.tile([P, 2, SG], f32, tag="att",
                                     name=f"att{sg}_{i}")
                    for h_ in range(2):
                        t = 2 * i + h_
                        nc.tensor.matmul(
                            ps[:, h_, :],
                            qkT_sb[:, 1, t * P:(t + 1) * P],
                            qkT_sb[:, 0, s_sl],
                            start=True,
                            stop=True,
                        )
                    expT = work_pool.tile([P, 2, SG], f16, tag="exp",
                                          name=f"exp{sg}_{i}", bufs=17)
                    nc.scalar.activation(expT, ps, Exp, scale=SCALE)
                    attd = work_pool.tile([P, 2, SG], f16, tag="attd",
                                          name=f"attd{sg}_{i}", bufs=17)
                    nc.vector.tensor_mul(
                        out=attd, in0=expT,
                        in1=keeps[sg][:, 2 * i:2 * i + 2, :],
                    )
                    st['expTs'][i] = expT
                    st['attds'][i] = attd

                def emit_av(st, i):
                    attd = st['attds'].pop(i)
                    for h_ in range(2):
                        t = 2 * i + h_
                        nc.tensor.matmul(
                            st['out'],
                            v_sb[:, t, :],
                            attd[:, h_, :],
                            start=(t == 0),
                            stop=(t == NT - 1),
                        )

                def emit_wave(st, w):
                    e0 = st['expTs'].pop(2 * w)
                    e1 = st['expTs'].pop(2 * w + 1)
                    for j in range(4):
                        src = (e0 if j < 2 else e1)[:, j % 2, :]
                        nc.tensor.matmul(
                            st['den'][32 * j:32 * j + 1, :],
                            ones_t,
                            src,
                            start=(w == 0),
                            stop=(w == NPAIR // 2 - 1),
                            tile_position=(0, 32 * j),
                        )

                def emit_norm_a(sg):
                    # den eviction, emitted right after the last den wave so
                    # the select matmul's operand is ready well before it
                    # issues at the next block's start
                    st = state[sg]
                    st['den_all'] = work_pool.tile([P, SG], f16, tag="den_all",
                                                   name=f"den_all{sg}")
                    nc.vector.tensor_copy(st['den_all'], st['den'])

                def emit_norm_b(sg):
                    st = state[sg]
                    s_sl = slice(sg * SG, (sg + 1) * SG)
                    nc.tensor.matmul(
                        st['den'], sel128, st['den_all'], start=True, stop=True)
                    recip_sb = work_pool.tile([P, SG], f32, tag="recip")
                    nc.vector.reciprocal_approx_fast(
                        out=recip_sb, in_=st['den'])
                    out_sb = work_pool.tile([P, SG], f16, tag="out_sb")
                    nc.vector.tensor_mul(
                        out=out_sb, in0=st['out'], in1=recip_sb)
                    nc.sync.dma_start(outT_d[:, s_sl], out_sb)
                    del state[sg]

                # v projection pieces, interleaved into s-group 0's pairs.
                # e consumed in DMA-arrival order; accumulators live in the
                # out/den banks (2 chains per pass).
                V_ORDER = [4, 5, 6, 7, 0, 1, 2, 3]

                def emit_v_part(i):
                    st0 = sg_state(0)
                    chains = [st0['out'], st0['den']]
                    half, step = divmod(i, 4)
                    for e2 in range(2):
                        ei = 2 * step + e2
                        e = V_ORDER[ei]
                        for c2 in range(2):
                            c = 2 * half + c2
                            nc.tensor.matmul(
                                chains[c2],
                                xwv_sb[:, e, S:S + H],
                                xwv_sb[:, e, c * SG:(c + 1) * SG],
                                start=(ei == 0),
                                stop=(ei == NE - 1),
                            )
                    if step == 3:
                        for c2 in range(2):
                            c = 2 * half + c2
                            nc.any.tensor_copy(
                                vT_sb[:, c * SG:(c + 1) * SG], chains[c2])

                def emit_v_transposes():
                    st0 = sg_state(0)
                    for g in range(NSG):
                        bank = st0['out'] if g % 2 else st0['den']
                        trv = bank[:].bitcast(f16)
                        for j in range(4):
                            nc.tensor.transpose(
                                trv[:, j * P:(j + 1) * P],
                                vT_sb[:, (4 * g + j) * P:(4 * g + j + 1) * P],
                                identity16,
                            )
                        nc.any.tensor_copy(
                            v_sb[:, 4 * g:4 * g + 4, :], trv[:, 0:4 * P])

                # blocks: pairs(n) with post(n-1) interleaved; each sg's
                # normalize tail (norm_b) lands at the start of the block
                # after its den waves completed (norm_a)
                for n in range(NSG):
                    prev = state.get(n - 1)
                    for i in range(NPAIR):
                        emit_pair(n, i)
                        if n == 0:
                            emit_v_part(i)
                        else:
                            if i == 0:
                                if n >= 2:
                                    emit_norm_b(n - 2)
                                nc.vector.memset(prev['den'], 0.0)
                            emit_av(prev, i)
                            if i % 2 == 1:
                                emit_wave(prev, (i - 1) // 2)
                    if n == 0:
                        emit_v_transposes()
                    else:
                        emit_norm_a(n - 1)

                # tail: finish s-group 3 (its out/den banks are free only
                # after s-group 2's normalize)
                emit_norm_b(NSG - 2)
                st3 = sg_state(NSG - 1)
                nc.vector.memset(st3['den'], 0.0)
                for i in range(NPAIR):
                    emit_av(st3, i)
                    if i % 2 == 1:
                        emit_wave(st3, (i - 1) // 2)
                emit_norm_a(NSG - 1)
                emit_norm_b(NSG - 1)

    nc.compile()
    _program_cache[key] = nc
    return nc


def kernel(x, wq, wk, wv, drop_u):
    from concourse import bass_utils

    x = np.asarray(x)
    wq = np.asarray(wq)
    wk = np.asarray(wk)
    wv = np.asarray(wv)
    drop_u = np.asarray(drop_u)

    nc = _build_program()
    in_maps = build_in_maps(x, wq, wk, wv, drop_u)
    last_err = None
    for _attempt in range(3):
        try:
            res = bass_utils.run_bass_kernel_spmd(
                nc, in_maps, core_ids=list(range(B)), trace=False
            )
            return np.stack(
                [np.asarray(res.results[b]["outT"]).T.astype(np.float32)
                 for b in range(B)],
                axis=0,
            )
        except Exception as e:  # transient device errors — retry
            last_err = e
            import time as _time

            _time.sleep(2.0)
    raise last_err


def _arrange_pe(a, ne):
    """[E, N] -> [128, ne, N] with e-chunk rows contiguous per partition."""
    E_, N_ = a.shape
    return np.ascontiguousarray(a.reshape(ne, P, N_).transpose(1, 0, 2))


def build_in_maps(x, wq, wk, wv, drop_u):
    f8 = ml_dtypes.float8_e4m3
    NE = E // P
    NT = S // P
    wq8 = _arrange_pe((np.asarray(wq) * W_SCALE).astype(f8), NE)
    wk8 = _arrange_pe((np.asarray(wk) * W_SCALE).astype(f8), NE)
    wv16 = _arrange_pe(np.asarray(wv).astype(np.float16), NE)
    in_maps = []
    for b in range(B):
        xTb = np.ascontiguousarray(x[b].T)
        x8 = _arrange_pe(xTb.astype(f8), NE)
        xT = _arrange_pe(xTb.astype(np.float16), NE)
        xw8 = np.concatenate([x8, wq8, wk8], axis=2)
        xwv = np.concatenate([xT, wv16], axis=2)
        keep = _arrange_pe(
            (drop_u[b].T >= np.float32(DROP_P)).astype(np.float16), NT)
        in_maps.append({"xw8": xw8, "xwv": xwv, "keep": keep})
    return in_maps


# revision 47
# speedup vs baseline: 1.0468x; 1.0468x over previous
"""Single-head attention (B=8, S=2048, E=1024, H=128) with softmax + deterministic
dropout, data-parallel over batch across 8 NeuronCores (one batch element per core).

Layout ("transposed attention"), engineered so the ACT engine (which does the
4M-element exp() per core, the hard throughput floor at ~27us) never bubbles:

  - host ships x with the projection weights CONCATENATED per e-chunk row:
    xw8 fp8e4m3 [128, NE, S+2H] = x8 rows ++ wq8 ++ wk8 (weights pre-scaled by
    32 for fp8 subnormal avoidance, folded into the exp scale), and
    xwv fp16 [128, NE, S+H] = xT rows ++ wv.  One contiguous stream per dtype:
    no separate (slow) small weight DMAs, and the first q,k matmul can start
    as soon as the first e-pair lands.
  - q,k projections are fp8 DoubleRow matmuls (K=256/instruction, 2x PE rate);
    v projection is fp16.
  - dropout mask shipped {0,1} fp16 (keeps the wide DVE multiply in 2x 16-bit
    mode; the 1/(1-p) factor is folded into the denominator select matmul).
    The mask/xwv/output queue (Sync) is GATED behind the first q,k eviction
    via a dummy copy into the first mask tile, so it cannot steal HBM
    bandwidth from the projection-critical fp8 stream at startup.
  - attention is one flat stream of t-pairs: per pair one 2-bank PSUM tile
    takes two QK matmuls, one wide ACT exp -> expT fp16, one wide DVE mask
    multiply.  Each s-group's post-work (denominator waves, AV matmuls,
    normalize) is INTERLEAVED into the NEXT s-group's pair emission, so the
    PE always has exp-feeding pair matmuls in flight and ACT stays saturated
    across s-group boundaries.  The v projection is interleaved into
    s-group 0's pairs the same way, using the out/den PSUM banks (idle until
    the first AV matmul) for its accumulators and transpose scratch.
  - denominator: M=1 ones-matmuls per expT chunk in waves of 4 packed into
    distinct PE column groups (tile_position), emitted when operands are
    long-ready so they issue back-to-back and overlap in the PE array.
  - normalize: den partials in PSUM rows {0,32,64,96}; one select matmul sums
    and broadcasts 0.9*den[s]; reciprocal_approx_fast + one DVE multiply
    rescale out^T during eviction.  Output stays [H, S] fp16; the host
    transposes back to [S, H] fp32.

PSUM (8 banks): 3 x [128,2,512] QK tiles (6) + out (1) + den (1), with the v
projection and its transpose scratch time-sharing out/den before the AVs.
"""

import sys

for _p in ("/opt/trn_rl_repo",):
    if _p not in sys.path:
        sys.path.append(_p)

import numpy as np
import ml_dtypes

B, S, E, H = 8, 2048, 1024, 128
DROP_P = 0.1
P = 128
W_SCALE = 32.0  # host premultiplies wq/wk by this before fp8 cast

_program_cache = {}


def _build_program(S=S, E=E):
    key = (S, E)
    if key in _program_cache:
        return _program_cache[key]
    NT = S // P       # t-chunks (16)
    NE = E // P       # e-chunks (8)
    NEP = NE // 2     # e-pairs for DoubleRow (4)
    SG = 512          # s-group width (one fp32 PSUM bank)
    NSG = S // SG     # 4
    NPAIR = NT // 2   # t-pairs per s-group (8)
    W8 = S + 2 * H    # xw8 row length
    WV = S + H        # xwv row length

    import concourse.bass as bass  # noqa: F401
    import concourse.mybir as mybir
    import concourse.tile as tile
    from concourse import bacc
    from concourse.masks import make_identity

    f32 = mybir.dt.float32
    f16 = mybir.dt.float16
    f8 = mybir.dt.float8e4
    Exp = mybir.ActivationFunctionType.Exp
    DR = mybir.MatmulPerfMode.DoubleRow
    SCALE = float(E) ** -0.5 / (W_SCALE * W_SCALE)

    nc = bacc.Bacc("TRN2", target_bir_lowering=False, debug=False)
    xw8_d = nc.dram_tensor("xw8", [P, NE, W8], f8, kind="ExternalInput").ap()
    xwv_d = nc.dram_tensor("xwv", [P, NE, WV], f16, kind="ExternalInput").ap()
    keep_d = nc.dram_tensor("keep", [P, NT, S], f16, kind="ExternalInput").ap()
    outT_d = nc.dram_tensor("outT", [H, S], f16, kind="ExternalOutput").ap()

    with tile.TileContext(nc) as tc:
        with (
            tc.tile_pool(name="consts", bufs=1) as consts,
            tc.tile_pool(name="xw", bufs=1) as xw_pool,
            tc.tile_pool(name="keep_pool", bufs=2) as keep_pool,
        ):
            identity16 = consts.tile([P, P], f16)
            make_identity(nc, identity16)
            ones_t = consts.tile([P, 1], f16)
            nc.vector.memset(ones_t, 1.0)
            sel128 = consts.tile([P, P], f16)
            nc.vector.memset(sel128, 0.0)
            for j in range(4):
                nc.vector.memset(sel128[32 * j:32 * j + 1, :], 1.0 - DROP_P)

            xw8_sb = xw_pool.tile([P, NE, W8], f8, tag="xw8", name="xw8")
            xwv_sb = xw_pool.tile([P, NE, WV], f16, tag="xwv", name="xwv")
            qkT_sb = xw_pool.tile([P, 2, S], f16, tag="qkT", name="qkT")
            vT_sb = xw_pool.tile([P, S], f16, tag="vT", name="vT")
            v_sb = xw_pool.tile([P, NT, H], f16, tag="v", name="v")

            # qAct (scalar) queue: the projection-critical fp8 stream, then
            # the tail of xwv.  qSP starts with the head of xwv (small, needed
            # early by the v projection) — the mask fetches behind it are
            # gated below.
            for ep in range(NEP):
                nc.scalar.dma_start(
                    xw8_sb[:, 2 * ep:2 * ep + 2, :], xw8_d[:, 2 * ep:2 * ep + 2, :]
                )
            nc.scalar.dma_start(
                xwv_sb[:, 6:8, :], xwv_d[:, 6:8, :]
            )

            keeps = {}

            def fetch_keep(sg, engine, half=None):
                if sg not in keeps:
                    keeps[sg] = keep_pool.tile([P, NT, SG], f16, tag="keep",
                                               name=f"keep{sg}")
                t_ = keeps[sg]
                sl = slice(sg * SG, (sg + 1) * SG)
                if half is None:
                    engine.dma_start(t_, keep_d[:, :, sl])
                else:
                    rows = slice(0, NT // 2) if half == 0 else slice(NT // 2, NT)
                    engine.dma_start(t_[:, rows, :], keep_d[:, rows, sl])

            # -------- q,k projections: fp8 DoubleRow, e-pair-major --------
            with tc.tile_pool(name="proj_ps", bufs=8, space="PSUM") as proj_ps:
                ps_qk = [
                    proj_ps.tile([P, SG], f32, tag=f"pqk{j}{c}",
                                 name=f"pqk{j}{c}", bufs=1)
                    for j in range(2) for c in range(NSG)
                ]
                for ep in range(NEP):
                    for j in range(2):
                        for c in range(NSG):
                            nc.tensor.matmul(
                                ps_qk[j * NSG + c],
                                xw8_sb[:, 2 * ep:2 * ep + 2,
                                       S + j * H:S + (j + 1) * H],
                                xw8_sb[:, 2 * ep:2 * ep + 2, c * SG:(c + 1) * SG],
                                start=(ep == 0),
                                stop=(ep == NEP - 1),
                                perf_mode=DR,
                            )
                for j in range(2):
                    for c in range(NSG):
                        nc.any.tensor_copy(
                            qkT_sb[:, j, c * SG:(c + 1) * SG],
                            ps_qk[j * NSG + c],
                        )

            # Gate the Sync queue behind the first q,k eviction: emitted
            # AFTER the eviction, this copy reads qkT (RAW dep) and writes a
            # corner of the first mask tile, so the mask DMA — and everything
            # queued after it on the Sync queue — cannot steal HBM bandwidth
            # from the projection-critical fp8 stream.
            # qSP queue: first mask half, xwv head (the v projection wants
            # it early), then the remaining mask fetches.  Empirically this
            # ordering beats gating the mask stream behind the projection:
            # the HBM arbiter shares bandwidth well enough, and a late mask
            # stalls the DVE multiply FIFO, which cascades.
            fetch_keep(0, nc.sync, half=0)
            for ep in (0, 1, 2):
                nc.sync.dma_start(
                    xwv_sb[:, 2 * ep:2 * ep + 2, :], xwv_d[:, 2 * ep:2 * ep + 2, :]
                )
            fetch_keep(0, nc.sync, half=1)
            fetch_keep(1, nc.sync)
            fetch_keep(2, nc.sync)
            fetch_keep(3, nc.sync)

            # -------- flat attention pipeline --------
            with (
                tc.tile_pool(name="att_ps", bufs=3, space="PSUM") as att_ps,
                tc.tile_pool(name="acc_ps", bufs=1, space="PSUM") as acc_ps,
                tc.tile_pool(name="work", bufs=2) as work_pool,
            ):
                state = {}

                def sg_state(sg):
                    if sg not in state:
                        state[sg] = dict(
                            out=acc_ps.tile([P, SG], f32, tag="out",
                                            name=f"out{sg}"),
                            den=acc_ps.tile([P, SG], f32, tag="den",
                                            name=f"den{sg}"),
                            expTs={}, attds={},
                        )
                    return state[sg]

                def emit_pair(sg, i):
                    st = sg_state(sg)
                    s_sl = slice(sg * SG, (sg + 1) * SG)
                    ps = att_ps.tile([P, 2, SG], f32, tag="att",
                                     name=f"att{sg}_{i}")
                    for h_ in range(2):
                        t = 2 * i + h_
                        nc.tensor.matmul(
                            ps[:, h_, :],
                            qkT_sb[:, 1, t * P:(t + 1) * P],
                            qkT_sb[:, 0, s_sl],
                            start=True,
                            stop=True,
                        )
                    expT = work_pool.tile([P, 2, SG], f16, tag="exp",
                                          name=f"exp{sg}_{i}", bufs=17)
                    nc.scalar.activation(expT, ps, Exp, scale=SCALE)
                    attd = work_pool.tile([P, 2, SG], f16, tag="attd",
                                          name=f"attd{sg}_{i}", bufs=17)
                    nc.vector.tensor_mul(
                        out=attd, in0=expT,
                        in1=keeps[sg][:, 2 * i:2 * i + 2, :],
                    )
                    st['expTs'][i] = expT
                    st['attds'][i] = attd

                def emit_av(st, i):
                    attd = st['attds'].pop(i)
                    for h_ in range(2):
                        t = 2 * i + h_
                        nc.tensor.matmul(
                            st['out'],
                            v_sb[:, t, :],
                            attd[:, h_, :],
                            start=(t == 0),
                            stop=(t == NT - 1),
                        )

                def emit_wave(st, w):
                    e0 = st['expTs'].pop(2 * w)
                    e1 = st['expTs'].pop(2 * w + 1)
                    for j in range(4):
                        src = (e0 if j < 2 else e1)[:, j % 2, :]
                        nc.tensor.matmul(
                            st['den'][32 * j:32 * j + 1, :],
                            ones_t,
                            src,
                            start=(w == 0),
                            stop=(w == NPAIR // 2 - 1),
                            tile_position=(0, 32 * j),
                        )

                def emit_norm_a(sg):
                    # den eviction, emitted right after the last den wave so
                    # the select matmul's operand is ready well before it
                    # issues at the next block's start
                    st = state[sg]
                    st['den_all'] = work_pool.tile([P, SG], f16, tag="den_all",
                                                   name=f"den_all{sg}")
                    nc.vector.tensor_copy(st['den_all'], st['den'])

                def emit_norm_b(sg):
                    st = state[sg]
                    s_sl = slice(sg * SG, (sg + 1) * SG)
                    nc.tensor.matmul(
                        st['den'], sel128, st['den_all'], start=True, stop=True)
                    recip_sb = work_pool.tile([P, SG], f32, tag="recip")
                    nc.vector.reciprocal_approx_fast(
                        out=recip_sb, in_=st['den'])
                    out_sb = work_pool.tile([P, SG], f16, tag="out_sb")
                    nc.vector.tensor_mul(
                        out=out_sb, in0=st['out'], in1=recip_sb)
                    nc.sync.dma_start(outT_d[:, s_sl], out_sb)
                    del state[sg]

                # v projection pieces, interleaved into s-group 0's pairs.
                # e consumed in DMA-arrival order; accumulators live in the
                # out/den banks (2 chains per pass).
                V_ORDER = [0, 1, 2, 3, 6, 7, 4, 5]

                def emit_v_part(i):
                    st0 = sg_state(0)
                    chains = [st0['out'], st0['den']]
                    half, step = divmod(i, 4)
                    for e2 in range(2):
                        ei = 2 * step + e2
                        e = V_ORDER[ei]
                        for c2 in range(2):
                            c = 2 * half + c2
                            nc.tensor.matmul(
                                chains[c2],
                                xwv_sb[:, e, S:S + H],
                                xwv_sb[:, e, c * SG:(c + 1) * SG],
                                start=(ei == 0),
                                stop=(ei == NE - 1),
                            )
                    if step == 3:
                        for c2 in range(2):
                            c = 2 * half + c2
                            nc.any.tensor_copy(
                                vT_sb[:, c * SG:(c + 1) * SG], chains[c2])

                def emit_v_transposes():
                    st0 = sg_state(0)
                    for g in range(NSG):
                        bank = st0['out'] if g % 2 else st0['den']
                        trv = bank[:].bitcast(f16)
                        for j in range(4):
                            nc.tensor.transpose(
                                trv[:, j * P:(j + 1) * P],
                                vT_sb[:, (4 * g + j) * P:(4 * g + j + 1) * P],
                                identity16,
                            )
                        nc.any.tensor_copy(
                            v_sb[:, 4 * g:4 * g + 4, :], trv[:, 0:4 * P])

                # blocks: pairs(n) with post(n-1) interleaved; each sg's
                # normalize tail (norm_b) lands at the start of the block
                # after its den waves completed (norm_a)
                for n in range(NSG):
                    prev = state.get(n - 1)
                    for i in range(NPAIR):
                        emit_pair(n, i)
                        if n == 0:
                            emit_v_part(i)
                        else:
                            if i == 0:
                                if n >= 2:
                                    emit_norm_b(n - 2)
                                nc.vector.memset(prev['den'], 0.0)
                            emit_av(prev, i)
                            if i % 2 == 1:
                                emit_wave(prev, (i - 1) // 2)
                    if n == 0:
                        emit_v_transposes()
                    else:
                        emit_norm_a(n - 1)

                # tail: finish s-group 3 (its out/den banks are free only
                # after s-group 2's normalize)
                emit_norm_b(NSG - 2)
                st3 = sg_state(NSG - 1)
                nc.vector.memset(st3['den'], 0.0)
                for i in range(NPAIR):
                    emit_av(st3, i)
                    if i % 2 == 1:
                        emit_wave(st3, (i - 1) // 2)
                emit_norm_a(NSG - 1)
                emit_norm_b(NSG - 1)

    nc.compile()
    _program_cache[key] = nc
    return nc


def kernel(x, wq, wk, wv, drop_u):
    from concourse import bass_utils

    x = np.asarray(x)
    wq = np.asarray(wq)
    wk = np.asarray(wk)
    wv = np.asarray(wv)
    drop_u = np.asarray(drop_u)

    nc = _build_program()
    in_maps = build_in_maps(x, wq, wk, wv, drop_u)
    last_err = None
    for _attempt in range(3):
        try:
            res = bass_utils.run_bass_kernel_spmd(
                nc, in_maps, core_ids=list(range(B)), trace=False
            )
            return np.stack(
                [np.asarray(res.results[b]["outT"]).T.astype(np.float32)
                 for b in range(B)],
                axis=0,
            )
        except Exception as e:  # transient device errors — retry
            last_err = e
            import time as _time

            _time.sleep(2.0)
    raise last_err


def _arrange_pe(a, ne):
    """[E, N] -> [128, ne, N] with e-chunk rows contiguous per partition."""
    E_, N_ = a.shape
    return np.ascontiguousarray(a.reshape(ne, P, N_).transpose(1, 0, 2))


def build_in_maps(x, wq, wk, wv, drop_u):
    f8 = ml_dtypes.float8_e4m3
    NE = E // P
    NT = S // P
    wq8 = _arrange_pe((np.asarray(wq) * W_SCALE).astype(f8), NE)
    wk8 = _arrange_pe((np.asarray(wk) * W_SCALE).astype(f8), NE)
    wv16 = _arrange_pe(np.asarray(wv).astype(np.float16), NE)
    in_maps = []
    for b in range(B):
        xTb = np.ascontiguousarray(x[b].T)
        x8 = _arrange_pe(xTb.astype(f8), NE)
        xT = _arrange_pe(xTb.astype(np.float16), NE)
        xw8 = np.concatenate([x8, wq8, wk8], axis=2)
        xwv = np.concatenate([xT, wv16], axis=2)
        keep = _arrange_pe(
            (drop_u[b].T >= np.float32(DROP_P)).astype(np.float16), NT)
        in_maps.append({"xw8": xw8, "xwv": xwv, "keep": keep})
    return in_maps


# revision 48
# speedup vs baseline: 1.0525x; 1.0055x over previous
"""Single-head attention (B=8, S=2048, E=1024, H=128) with softmax + deterministic
dropout, data-parallel over batch across 8 NeuronCores (one batch element per core).

Layout ("transposed attention"), engineered so the ACT engine (which does the
4M-element exp() per core, the hard throughput floor at ~27us) never bubbles:

  - host ships x with the projection weights CONCATENATED per e-chunk row:
    xw8 fp8e4m3 [128, NE, S+2H] = x8 rows ++ wq8 ++ wk8 (weights pre-scaled by
    32 for fp8 subnormal avoidance, folded into the exp scale), and
    xwv fp16 [128, NE, S+H] = xT rows ++ wv.  One contiguous stream per dtype:
    no separate (slow) small weight DMAs, and the first q,k matmul can start
    as soon as the first e-pair lands.
  - q,k projections are fp8 DoubleRow matmuls (K=256/instruction, 2x PE rate);
    v projection is fp16.
  - dropout mask shipped {0,1} fp16 (keeps the wide DVE multiply in 2x 16-bit
    mode; the 1/(1-p) factor is folded into the denominator select matmul).
    The mask/xwv/output queue (Sync) is GATED behind the first q,k eviction
    via a dummy copy into the first mask tile, so it cannot steal HBM
    bandwidth from the projection-critical fp8 stream at startup.
  - attention is one flat stream of t-pairs: per pair one 2-bank PSUM tile
    takes two QK matmuls, one wide ACT exp -> expT fp16, one wide DVE mask
    multiply.  Each s-group's post-work (denominator waves, AV matmuls,
    normalize) is INTERLEAVED into the NEXT s-group's pair emission, so the
    PE always has exp-feeding pair matmuls in flight and ACT stays saturated
    across s-group boundaries.  The v projection is interleaved into
    s-group 0's pairs the same way, using the out/den PSUM banks (idle until
    the first AV matmul) for its accumulators and transpose scratch.
  - denominator: M=1 ones-matmuls per expT chunk in waves of 4 packed into
    distinct PE column groups (tile_position), emitted when operands are
    long-ready so they issue back-to-back and overlap in the PE array.
  - normalize: den partials in PSUM rows {0,32,64,96}; one select matmul sums
    and broadcasts 0.9*den[s]; reciprocal_approx_fast + one DVE multiply
    rescale out^T during eviction.  Output stays [H, S] fp16; the host
    transposes back to [S, H] fp32.

PSUM (8 banks): 3 x [128,2,512] QK tiles (6) + out (1) + den (1), with the v
projection and its transpose scratch time-sharing out/den before the AVs.
"""

import sys

for _p in ("/opt/trn_rl_repo",):
    if _p not in sys.path:
        sys.path.append(_p)

import numpy as np
import ml_dtypes

B, S, E, H = 8, 2048, 1024, 128
DROP_P = 0.1
P = 128
W_SCALE = 32.0  # host premultiplies wq/wk by this before fp8 cast

_program_cache = {}


def _build_program(S=S, E=E):
    key = (S, E)
    if key in _program_cache:
        return _program_cache[key]
    NT = S // P       # t-chunks (16)
    NE = E // P       # e-chunks (8)
    NEP = NE // 2     # e-pairs for DoubleRow (4)
    SG = 512          # s-group width (one fp32 PSUM bank)
    NSG = S // SG     # 4
    NPAIR = NT // 2   # t-pairs per s-group (8)
    W8 = S + 2 * H    # xw8 row length
    WV = S + H        # xwv row length

    import concourse.bass as bass  # noqa: F401
    import concourse.mybir as mybir
    import concourse.tile as tile
    from concourse import bacc
    from concourse.masks import make_identity

    f32 = mybir.dt.float32
    f16 = mybir.dt.float16
    f8 = mybir.dt.float8e4
    Exp = mybir.ActivationFunctionType.Exp
    DR = mybir.MatmulPerfMode.DoubleRow
    SCALE = float(E) ** -0.5 / (W_SCALE * W_SCALE)

    nc = bacc.Bacc("TRN2", target_bir_lowering=False, debug=False)
    xw8_d = nc.dram_tensor("xw8", [P, NE, W8], f8, kind="ExternalInput").ap()
    xwv_d = nc.dram_tensor("xwv", [P, NE, WV], f16, kind="ExternalInput").ap()
    keep_d = nc.dram_tensor("keep", [P, NT, S], f16, kind="ExternalInput").ap()
    outT_d = nc.dram_tensor("outT", [H, S], f16, kind="ExternalOutput").ap()

    with tile.TileContext(nc) as tc:
        with (
            tc.tile_pool(name="consts", bufs=1) as consts,
            tc.tile_pool(name="xw", bufs=1) as xw_pool,
            tc.tile_pool(name="keep_pool", bufs=2) as keep_pool,
        ):
            identity16 = consts.tile([P, P], f16)
            make_identity(nc, identity16)
            ones_t = consts.tile([P, 1], f16)
            nc.vector.memset(ones_t, 1.0)
            sel128 = consts.tile([P, P], f16)
            nc.vector.memset(sel128, 0.0)
            for j in range(4):
                nc.vector.memset(sel128[32 * j:32 * j + 1, :], 1.0 - DROP_P)

            xw8_sb = xw_pool.tile([P, NE, W8], f8, tag="xw8", name="xw8")
            xwv_sb = xw_pool.tile([P, NE, WV], f16, tag="xwv", name="xwv")
            qkT_sb = xw_pool.tile([P, 2, S], f16, tag="qkT", name="qkT")
            vT_sb = xw_pool.tile([P, S], f16, tag="vT", name="vT")
            v_sb = xw_pool.tile([P, NT, H], f16, tag="v", name="v")

            # qAct (scalar) queue: the projection-critical fp8 stream, then
            # the tail of xwv.  qSP starts with the head of xwv (small, needed
            # early by the v projection) — the mask fetches behind it are
            # gated below.
            for ep in range(NEP):
                nc.scalar.dma_start(
                    xw8_sb[:, 2 * ep:2 * ep + 2, :], xw8_d[:, 2 * ep:2 * ep + 2, :]
                )
            nc.scalar.dma_start(
                xwv_sb[:, 6:8, :], xwv_d[:, 6:8, :]
            )

            keeps = {}

            def fetch_keep(sg, engine, half=None):
                if sg not in keeps:
                    keeps[sg] = keep_pool.tile([P, NT, SG], f16, tag="keep",
                                               name=f"keep{sg}")
                t_ = keeps[sg]
                sl = slice(sg * SG, (sg + 1) * SG)
                if half is None:
                    engine.dma_start(t_, keep_d[:, :, sl])
                else:
                    rows = slice(0, NT // 2) if half == 0 else slice(NT // 2, NT)
                    engine.dma_start(t_[:, rows, :], keep_d[:, rows, sl])

            # -------- q,k projections: fp8 DoubleRow, e-pair-major --------
            with tc.tile_pool(name="proj_ps", bufs=8, space="PSUM") as proj_ps:
                ps_qk = [
                    proj_ps.tile([P, SG], f32, tag=f"pqk{j}{c}",
                                 name=f"pqk{j}{c}", bufs=1)
                    for j in range(2) for c in range(NSG)
                ]
                for ep in range(NEP):
                    for j in range(2):
                        for c in range(NSG):
                            nc.tensor.matmul(
                                ps_qk[j * NSG + c],
                                xw8_sb[:, 2 * ep:2 * ep + 2,
                                       S + j * H:S + (j + 1) * H],
                                xw8_sb[:, 2 * ep:2 * ep + 2, c * SG:(c + 1) * SG],
                                start=(ep == 0),
                                stop=(ep == NEP - 1),
                                perf_mode=DR,
                            )
                for j in range(2):
                    for c in range(NSG):
                        nc.any.tensor_copy(
                            qkT_sb[:, j, c * SG:(c + 1) * SG],
                            ps_qk[j * NSG + c],
                        )

            # Gate the Sync queue behind the first q,k eviction: emitted
            # AFTER the eviction, this copy reads qkT (RAW dep) and writes a
            # corner of the first mask tile, so the mask DMA — and everything
            # queued after it on the Sync queue — cannot steal HBM bandwidth
            # from the projection-critical fp8 stream.
            # qSP queue: first mask half, xwv head (the v projection wants
            # it early), then the remaining mask fetches.  Empirically this
            # ordering beats gating the mask stream behind the projection:
            # the HBM arbiter shares bandwidth well enough, and a late mask
            # stalls the DVE multiply FIFO, which cascades.
            fetch_keep(0, nc.sync, half=0)
            for ep in (0, 1, 2):
                nc.sync.dma_start(
                    xwv_sb[:, 2 * ep:2 * ep + 2, :], xwv_d[:, 2 * ep:2 * ep + 2, :]
                )
            fetch_keep(0, nc.sync, half=1)
            fetch_keep(1, nc.sync)
            fetch_keep(2, nc.sync)
            fetch_keep(3, nc.sync)

            # -------- flat attention pipeline --------
            with (
                tc.tile_pool(name="att_ps", bufs=3, space="PSUM") as att_ps,
                tc.tile_pool(name="acc_ps", bufs=1, space="PSUM") as acc_ps,
                tc.tile_pool(name="work", bufs=2) as work_pool,
            ):
                state = {}

                def sg_state(sg):
                    if sg not in state:
                        state[sg] = dict(
                            out=acc_ps.tile([P, SG], f32, tag="out",
                                            name=f"out{sg}"),
                            den=acc_ps.tile([P, SG], f32, tag="den",
                                            name=f"den{sg}"),
                            expTs={}, attds={},
                        )
                    return state[sg]

                def emit_pair(sg, i):
                    st = sg_state(sg)
                    s_sl = slice(sg * SG, (sg + 1) * SG)
                    ps = att_ps.tile([P, 2, SG], f32, tag="att",
                                     name=f"att{sg}_{i}")
                    for h_ in range(2):
                        t = 2 * i + h_
                        nc.tensor.matmul(
                            ps[:, h_, :],
                            qkT_sb[:, 1, t * P:(t + 1) * P],
                            qkT_sb[:, 0, s_sl],
                            start=True,
                            stop=True,
                        )
                    expT = work_pool.tile([P, 2, SG], f16, tag="exp",
                                          name=f"exp{sg}_{i}", bufs=17)
                    nc.scalar.activation(expT, ps, Exp, scale=SCALE)
                    attd = work_pool.tile([P, 2, SG], f16, tag="attd",
                                          name=f"attd{sg}_{i}", bufs=17)
                    nc.vector.tensor_mul(
                        out=attd, in0=expT,
                        in1=keeps[sg][:, 2 * i:2 * i + 2, :],
                    )
                    st['expTs'][i] = expT
                    st['attds'][i] = attd

                def emit_av(st, i):
                    attd = st['attds'].pop(i)
                    for h_ in range(2):
                        t = 2 * i + h_
                        nc.tensor.matmul(
                            st['out'],
                            v_sb[:, t, :],
                            attd[:, h_, :],
                            start=(t == 0),
                            stop=(t == NT - 1),
                        )

                def emit_wave(st, w):
                    e0 = st['expTs'].pop(2 * w)
                    e1 = st['expTs'].pop(2 * w + 1)
                    for j in range(4):
                        src = (e0 if j < 2 else e1)[:, j % 2, :]
                        nc.tensor.matmul(
                            st['den'][32 * j:32 * j + 1, :],
                            ones_t,
                            src,
                            start=(w == 0),
                            stop=(w == NPAIR // 2 - 1),
                            tile_position=(0, 32 * j),
                        )

                def emit_norm_a(sg):
                    # den eviction, emitted right after the last den wave so
                    # the select matmul's operand is ready well before it
                    # issues at the next block's start
                    st = state[sg]
                    st['den_all'] = work_pool.tile([P, SG], f16, tag="den_all",
                                                   name=f"den_all{sg}")
                    nc.vector.tensor_copy(st['den_all'], st['den'])

                def emit_norm_b(sg):
                    st = state[sg]
                    s_sl = slice(sg * SG, (sg + 1) * SG)
                    nc.tensor.matmul(
                        st['den'], sel128, st['den_all'], start=True, stop=True)
                    recip_sb = work_pool.tile([P, SG], f32, tag="recip")
                    nc.vector.reciprocal_approx_fast(
                        out=recip_sb, in_=st['den'])
                    out_sb = work_pool.tile([P, SG], f16, tag="out_sb")
                    nc.vector.tensor_mul(
                        out=out_sb, in0=st['out'], in1=recip_sb)
                    nc.sync.dma_start(outT_d[:, s_sl], out_sb)
                    del state[sg]

                # v projection pieces, interleaved into s-group 0's pairs.
                # e consumed in DMA-arrival order; accumulators live in the
                # out/den banks (2 chains per pass).
                V_ORDER = [0, 1, 2, 3, 6, 7, 4, 5]

                def emit_v_part(i):
                    st0 = sg_state(0)
                    chains = [st0['out'], st0['den']]
                    half, step = divmod(i, 4)
                    for e2 in range(2):
                        ei = 2 * step + e2
                        e = V_ORDER[ei]
                        for c2 in range(2):
                            c = 2 * half + c2
                            nc.tensor.matmul(
                                chains[c2],
                                xwv_sb[:, e, S:S + H],
                                xwv_sb[:, e, c * SG:(c + 1) * SG],
                                start=(ei == 0),
                                stop=(ei == NE - 1),
                            )
                    if step == 3:
                        for c2 in range(2):
                            c = 2 * half + c2
                            nc.any.tensor_copy(
                                vT_sb[:, c * SG:(c + 1) * SG], chains[c2])

                def emit_v_transposes():
                    st0 = sg_state(0)
                    for g in range(NSG):
                        bank = st0['out'] if g % 2 else st0['den']
                        trv = bank[:].bitcast(f16)
                        for j in range(4):
                            nc.tensor.transpose(
                                trv[:, j * P:(j + 1) * P],
                                vT_sb[:, (4 * g + j) * P:(4 * g + j + 1) * P],
                                identity16,
                            )
                        nc.any.tensor_copy(
                            v_sb[:, 4 * g:4 * g + 4, :], trv[:, 0:4 * P])

                # blocks: pairs(n) with post(n-1) interleaved; each sg's
                # normalize tail (norm_b) lands at the start of the block
                # after its den waves completed (norm_a)
                for n in range(NSG):
                    prev = state.get(n - 1)
                    for i in range(NPAIR):
                        emit_pair(n, i)
                        if n == 0:
                            emit_v_part(i)
                        else:
                            if i == 0:
                                if n >= 2:
                                    emit_norm_b(n - 2)
                                nc.vector.memset(prev['den'], 0.0)
                            emit_av(prev, i)
                            if i % 2 == 1:
                                emit_wave(prev, (i - 1) // 2)
                        if n == NSG - 1:
                            # s-group 3's AV matmuls accumulate into a freed
                            # QK-psum tile half (the out bank is still owned
                            # by s-group 2's pending normalize), so they
                            # overlap the last exps instead of trailing them
                            st3 = sg_state(n)
                            if i == 1:
                                out3 = att_ps.tile([P, 2, SG], f32, tag="att",
                                                   name="out3")
                                st3['out'] = out3[:, 0, :]
                            if i >= 1:
                                emit_av(st3, i - 1)
                    if n == 0:
                        emit_v_transposes()
                    else:
                        emit_norm_a(n - 1)

                # tail: finish s-group 3
                st3 = sg_state(NSG - 1)
                emit_av(st3, NPAIR - 1)
                emit_norm_b(NSG - 2)
                nc.vector.memset(st3['den'], 0.0)
                for w in range(NPAIR // 2):
                    emit_wave(st3, w)
                emit_norm_a(NSG - 1)
                emit_norm_b(NSG - 1)

    nc.compile()
    _program_cache[key] = nc
    return nc


def kernel(x, wq, wk, wv, drop_u):
    from concourse import bass_utils

    x = np.asarray(x)
    wq = np.asarray(wq)
    wk = np.asarray(wk)
    wv = np.asarray(wv)
    drop_u = np.asarray(drop_u)

    nc = _build_program()
    in_maps = build_in_maps(x, wq, wk, wv, drop_u)
    last_err = None
    for _attempt in range(3):
        try:
            res = bass_utils.run_bass_kernel_spmd(
                nc, in_maps, core_ids=list(range(B)), trace=False
            )
            return np.stack(
                [np.asarray(res.results[b]["outT"]).T.astype(np.float32)
                 for b in range(B)],
                axis=0,
            )
        except Exception as e:  # transient device errors — retry
            last_err = e
            import time as _time

            _time.sleep(2.0)
    raise last_err


def _arrange_pe(a, ne):
    """[E, N] -> [128, ne, N] with e-chunk rows contiguous per partition."""
    E_, N_ = a.shape
    return np.ascontiguousarray(a.reshape(ne, P, N_).transpose(1, 0, 2))


def build_in_maps(x, wq, wk, wv, drop_u):
    f8 = ml_dtypes.float8_e4m3
    NE = E // P
    NT = S // P
    wq8 = _arrange_pe((np.asarray(wq) * W_SCALE).astype(f8), NE)
    wk8 = _arrange_pe((np.asarray(wk) * W_SCALE).astype(f8), NE)
    wv16 = _arrange_pe(np.asarray(wv).astype(np.float16), NE)
    in_maps = []
    for b in range(B):
        xTb = np.ascontiguousarray(x[b].T)
        x8 = _arrange_pe(xTb.astype(f8), NE)
        xT = _arrange_pe(xTb.astype(np.float16), NE)
        xw8 = np.concatenate([x8, wq8, wk8], axis=2)
        xwv = np.concatenate([xT, wv16], axis=2)
        keep = _arrange_pe(
            (drop_u[b].T >= np.float32(DROP_P)).astype(np.float16), NT)
        in_maps.append({"xw8": xw8, "xwv": xwv, "keep": keep})
    return in_maps
